# revision 33
# baseline (speedup 1.0000x reference)
"""Trainium2 Bass kernel: MergedQKVParallelLinearWithLoRA.

out = x @ w_qkv.T + concat_s( lora_expand_s( lora_shrink_s(x)[token's lora] ) )

Strategy (8 NeuronCores, tensor-parallel on the merged QKV output dim):
  - Each core owns 768 of the 6144 output columns.
  - The LoRA is FOLDED into the base weights on the host:
    W_l = w_qkv + B_l @ A_l for each of the 8 loras (bf16).  Tokens are
    sorted by lora id host-side, so the device kernel is a pure grouped
    GEMM: each 128-token tile multiplies with the folded W of its lora.
    Per-group folded weights stream HBM->SBUF (6.3MB bf16 each) with a
    double-buffered pool, prefetched a full group ahead.
  - A 128-token tile that straddles a group boundary is computed with its
    majority segment's W, then corrected with a rank-96 low-rank update:
    ps = [A_wrong; A_main] @ x_tile^T (PE), zero the columns of tokens
    outside the wrong segment, expand with [B_wrong; -B_main].
  - All matmuls in bf16 (1 row/cycle, no fp32r >=256 free-dim constraint),
    fp32 PSUM accumulation; ~2.6e-3 max rel err vs the 2e-2 gate.
  - Cold start is DMA-bound (~150-250GB/s while queues ramp): the first
    two tiles are fused into one shared k-loop (halving W-arrival demand)
    and their x/W chunks are emitted on the sync queue in exact
    consumption order, which also FIFO-gates later-group W prefetches out
    of the starved window.

The kernel is specialized at build time to the token->lora grouping;
`kernel()` re-derives it from token_lora_idx on every call, so it is
correct for arbitrary inputs of the fixed shapes below.
"""

import numpy as np
import ml_dtypes

import concourse.mybir as mybir
import concourse.tile as tile
from concourse import bacc, bass_utils

T, D = 8192, 4096
L, R = 8, 16
OUT_SLICES = (4096, 1024, 1024)
O = sum(OUT_SLICES)          # 6144
NCORES = 8
OS = O // NCORES             # 768 output cols per core
P = 128
KT = D // P                  # 32 k-tiles
NT = T // P                  # 64 token tiles
RC = 3 * R                   # 48 stacked lora-rank rows (q,k,v)
RC2 = 2 * RC                 # 96: [wrong; main] stacked correction rank
N0 = 512                     # matmul free-dim split at the PSUM bank edge

F32 = mybir.dt.float32
BF16 = mybir.dt.bfloat16
NPBF16 = ml_dtypes.bfloat16

LAST_RESULT = None           # BassKernelResults of the most recent run


def _schedule(sorted_idx: np.ndarray):
    """Per-128-token-tile list of (lora, a, b) sub-ranges (a/b rel. to tile)."""
    tiles = []
    for t in range(NT):
        win = sorted_idx[t * P : (t + 1) * P]
        segs = []
        a = 0
        for i in range(1, P + 1):
            if i == P or win[i] != win[a]:
                segs.append((int(win[a]), a, i))
                a = i
        tiles.append(segs)
    return tiles


def _build(tiles, corr):
    mains = [max(segs, key=lambda s: s[2] - s[1])[0] for segs in tiles]
    groups = []                      # (lora, tile_start, tile_end)
    t0 = 0
    for t in range(1, NT + 1):
        if t == NT or mains[t] != mains[t0]:
            groups.append((mains[t0], t0, t))
            t0 = t

    corr_by_tile = {}                # tile -> [(bi, a, b)]
    for bi, (t, a, b, _lw, _lm) in enumerate(corr):
        corr_by_tile.setdefault(t, []).append((bi, a, b))
    nb = max(len(corr), 1)
    maxc = max([len(v) for v in corr_by_tile.values()] + [1])
    cbufs = maxc + 2

    nc = bacc.Bacc("TRN2", target_bir_lowering=False, debug=False,
                   num_devices=NCORES)
    d_x = nc.dram_tensor("xT", [NT, P, KT, P], BF16, kind="ExternalInput")
    d_w = nc.dram_tensor("wT", [L, P, KT, OS], BF16, kind="ExternalInput")
    d_ab = nc.dram_tensor("abT", [nb, P, KT, RC2], BF16, kind="ExternalInput")
    d_bb = nc.dram_tensor("bbT", [nb, RC2, OS], BF16, kind="ExternalInput")
    d_o = nc.dram_tensor("out", [T, OS], F32, kind="ExternalOutput")

    with tile.TileContext(nc) as tc:
        with (
            tc.tile_pool(name="wpool", bufs=2) as wpool,
            tc.tile_pool(name="xpool", bufs=4) as xpool,
            tc.tile_pool(name="abpool", bufs=cbufs) as abpool,
            tc.tile_pool(name="bbpool", bufs=cbufs) as bbpool,
            tc.tile_pool(name="sbpool", bufs=cbufs) as sbpool,
            tc.tile_pool(name="opool", bufs=3) as opool,
            tc.tile_pool(name="bpsum", bufs=4, space="PSUM") as bpsum,
        ):
            corr_tiles = {}          # bi -> (abt, bbt) live SBUF tiles
            xcache = {}              # tile -> prefetched x SBUF tile

            fetched_corr = set()

            def fetch_corrections(t):
                # scalar-engine triggers: never queue behind x/out DMAs
                if t in fetched_corr:
                    return
                fetched_corr.add(t)
                for (bi, _a, _b) in corr_by_tile.get(t, ()):
                    abt = abpool.tile([P, KT, RC2], BF16, tag="ab")
                    nc.scalar.dma_start(abt[:], d_ab[bi])
                    bbt = bbpool.tile([RC2, OS], BF16, tag="bb")
                    nc.scalar.dma_start(bbt[:], d_bb[bi])
                    corr_tiles[bi] = (abt, bbt)

            def ensure_x(t):
                if t < NT and t not in xcache:
                    xtr = xpool.tile([P, KT, P], BF16, tag="xt")
                    nc.sync.dma_start(xtr[:], d_x[t])
                    xcache[t] = xtr

            def base_mm(pb, xtr, wt, k, st, sp):
                nc.tensor.matmul(pb[:, 0:N0], xtr[:, k, :],
                                 wt[:, k, 0:N0], start=st, stop=sp)
                nc.tensor.matmul(pb[:, N0:OS], xtr[:, k, :],
                                 wt[:, k, N0:OS], start=st, stop=sp)

            def drain(pb, t):
                ob = opool.tile([P, OS], F32, tag="ob")
                nc.vector.tensor_copy(ob[:], pb[:])
                nc.sync.dma_start(d_o[t * P : (t + 1) * P, :], ob[:])

            fetch_corrections(0)
            # Fuse the first n_cold tiles of group 0 into one shared k-loop:
            # each W k-chunk then feeds n_cold x the PE work, matching
            # W-arrival demand to the ~250GB/s cold-start DMA rate while the
            # 6.3MB W0 streams in.
            g0len = groups[0][2] - groups[0][1]
            n_cold = 0
            for n in (2,):
                if g0len >= n and not any(
                    corr_by_tile.get(groups[0][1] + j) for j in range(n)
                ):
                    n_cold = n
                    break
            for gi, (gl, gt0, gt1) in enumerate(groups):
                wt = wpool.tile([P, KT, OS], BF16, tag="wt")
                t = gt0
                if gi == 0 and n_cold:
                    # Cold start: everything on the sync queue in exact
                    # need-order — x chunks for the fused tiles first, W0
                    # k-progressively interleaved. Later-group W DMAs queue
                    # behind this stream, so nothing steals cold bandwidth.
                    xs = [xpool.tile([P, KT, P], BF16, tag="xt",
                                     name=f"xcold_{j}")
                          for j in range(n_cold)]
                    # uniform consumption-order interleave: per round,
                    # [xa, xb, W] k-chunks — arrival tracks the fused
                    # k-loop's demand with no bursts. A small first round
                    # gets the first matmul issued sooner.
                    bounds = (0, 2, 6, 10, 14, 18, 22, 26, 30, KT)
                    for k0, k1 in zip(bounds[:-1], bounds[1:]):
                        for j, xj in enumerate(xs):
                            nc.sync.dma_start(xj[:, k0:k1, :],
                                              d_x[gt0 + j, :, k0:k1, :])
                        nc.sync.dma_start(wt[:, k0:k1, :],
                                          d_w[gl, :, k0:k1, :])
                    for j in range(1, n_cold + 1):
                        fetch_corrections(gt0 + j)
                    pbs = [bpsum.tile([P, OS], F32, tag="pb",
                                      name=f"pbcold_{j}")
                           for j in range(n_cold)]
                    for k in range(KT):
                        st, sp = k == 0, k == KT - 1
                        for xj, pj in zip(xs, pbs):
                            base_mm(pj, xj, wt, k, st, sp)
                    for j, pj in enumerate(pbs):
                        drain(pj, gt0 + j)
                    t = gt0 + n_cold
                elif gi == 0:
                    for k0 in range(0, KT, 4):
                        nc.sync.dma_start(wt[:, k0 : k0 + 4, :],
                                          d_w[gl, :, k0 : k0 + 4, :])
                else:
                    for k0 in range(0, KT, 8):
                        nc.sync.dma_start(wt[:, k0 : k0 + 8, :],
                                          d_w[gl, :, k0 : k0 + 8, :])

                while t < gt1:
                    # Pair plain tiles in one shared k-loop: every LDWEIGHTS
                    # gets >=213ns of matmul shadow (vs zero slack behind the
                    # 256-col matmul). bufs=4 makes both accumulators fresh
                    # while the previous pair drains — no PSUM stall.
                    if (t + 1 < gt1 and t + 1 != NT - 1
                            and not corr_by_tile.get(t)
                            and not corr_by_tile.get(t + 1)):
                        ensure_x(t)
                        ensure_x(t + 1)
                        xa = xcache.pop(t)
                        xb = xcache.pop(t + 1)
                        fetch_corrections(t + 2)
                        fetch_corrections(t + 3)
                        pba = bpsum.tile([P, OS], F32, tag="pb")
                        pbb = bpsum.tile([P, OS], F32, tag="pb")
                        for k in range(KT):
                            st, sp = k == 0, k == KT - 1
                            base_mm(pba, xa, wt, k, st, sp)
                            base_mm(pbb, xb, wt, k, st, sp)
                        drain(pba, t)
                        drain(pbb, t + 1)
                        t += 2
                        continue

                    ensure_x(t)
                    xtr = xcache.pop(t)
                    if t + 1 < NT:
                        fetch_corrections(t + 1)

                    cs = corr_by_tile.get(t, ())
                    # Boundary shrinks first: their DVE zero-pad+copy then
                    # overlaps with the base k-loop below.
                    sbs = []
                    for (bi, a, b) in cs:
                        abt, bbt = corr_tiles.pop(bi)
                        a2, b2 = a & ~1, min(P, (b + 1) & ~1)
                        ps = bpsum.tile([RC2, P], F32, tag="pb")
                        for k in range(KT):
                            nc.tensor.matmul(
                                ps[:, a2:b2], abt[:, k, :], xtr[:, k, a2:b2],
                                start=(k == 0), stop=(k == KT - 1),
                            )
                        sb = sbpool.tile([RC2, P], BF16, tag="sb")
                        nc.vector.memset(sb[:], 0.0)
                        nc.vector.tensor_copy(sb[:, a:b], ps[:, a:b])
                        sbs.append((sb, bbt))

                    pb = bpsum.tile([P, OS], F32, tag="pb")
                    last_base = len(sbs) == 0
                    for k in range(KT):
                        base_mm(pb, xtr, wt, k, k == 0,
                                last_base and k == KT - 1)
                    for ci, (sb, bbt) in enumerate(sbs):
                        sp = ci == len(sbs) - 1
                        nc.tensor.matmul(pb[:, 0:N0], sb[:], bbt[:, 0:N0],
                                         start=False, stop=sp)
                        nc.tensor.matmul(pb[:, N0:OS], sb[:], bbt[:, N0:OS],
                                         start=False, stop=sp)

                    if t == NT - 1:
                        # tail trim: half-drains on two DMA queues so the
                        # final HBM write starts while the second PSUM half
                        # is still being copied out.
                        ob = opool.tile([P, OS], F32, tag="ob")
                        nc.vector.tensor_copy(ob[:, 0:N0], pb[:, 0:N0])
                        nc.sync.dma_start(d_o[t * P : (t + 1) * P, 0:N0],
                                          ob[:, 0:N0])
                        nc.vector.tensor_copy(ob[:, N0:OS], pb[:, N0:OS])
                        nc.gpsimd.dma_start(d_o[t * P : (t + 1) * P, N0:OS],
                                            ob[:, N0:OS])
                    else:
                        drain(pb, t)
                    t += 1

    nc.compile()
    return nc


def _prep(x, w_qkv, lora_a, lora_b_q, lora_b_k, lora_b_v, perm, tiles, corr):
    x = np.asarray(x, dtype=np.float32)
    xs = x[perm]
    # xT[t, p, kt, i] = xs[t*128+i, kt*128+p]
    xT = np.ascontiguousarray(
        xs.T.reshape(KT, P, NT, P).transpose(2, 1, 0, 3)
    ).astype(NPBF16)

    # a_cat[l] = [48, D] (q,k,v stacked); b_cat[l] = [48, O] zero-padded
    a_cat = np.ascontiguousarray(
        np.asarray(lora_a, np.float32).transpose(1, 0, 2, 3)
    ).reshape(L, RC, D)
    b_cat = np.zeros((L, RC, O), np.float32)
    off = 0
    for s, (bs, osz) in enumerate(
        zip((lora_b_q, lora_b_k, lora_b_v), OUT_SLICES)
    ):
        b_cat[:, R * s : R * (s + 1), off : off + osz] = np.asarray(
            bs, np.float32
        ).transpose(0, 2, 1)
        off += osz

    w_qkv = np.asarray(w_qkv, np.float32)
    w_shards = []                    # per core: [L, P, KT, OS] bf16 folded
    for c in range(NCORES):
        wc = w_qkv[c * OS : (c + 1) * OS]            # [OS, D]
        bc = b_cat[:, :, c * OS : (c + 1) * OS]      # [L, 48, OS]
        sh = np.empty((L, P, KT, OS), NPBF16)
        for l in range(L):
            wf = wc + bc[l].T @ a_cat[l]             # [OS, D]
            sh[l] = wf.T.reshape(KT, P, OS).transpose(1, 0, 2).astype(NPBF16)
        w_shards.append(sh)

    nb = max(len(corr), 1)
    abT = np.zeros((nb, P, KT, RC2), NPBF16)
    bbs = [np.zeros((nb, RC2, OS), NPBF16) for _ in range(NCORES)]
    for bi, (t, a, b, lw, lm) in enumerate(corr):
        A2 = np.concatenate([a_cat[lw], a_cat[lm]], axis=0)   # [96, D]
        abT[bi] = A2.T.reshape(KT, P, RC2).transpose(1, 0, 2).astype(NPBF16)
        for c in range(NCORES):
            bbs[c][bi, 0:RC] = b_cat[lw, :, c * OS : (c + 1) * OS].astype(NPBF16)
            bbs[c][bi, RC:RC2] = (-b_cat[lm, :, c * OS : (c + 1) * OS]).astype(NPBF16)
    return xT, w_shards, abT, bbs


def kernel(x, w_qkv, lora_a, lora_b_q, lora_b_k, lora_b_v, token_lora_idx):
    global LAST_RESULT
    idx = np.asarray(token_lora_idx)
    perm = np.argsort(idx, kind="stable")
    tiles = _schedule(idx[perm])
    corr = []                        # (tile, a, b, lora_wrong, lora_main)
    for t, segs in enumerate(tiles):
        lm = max(segs, key=lambda s: s[2] - s[1])[0]
        for (l, a, b) in segs:
            if l != lm:
                corr.append((t, a, b, l, lm))

    nc = _build(tiles, corr)
    xT, w_shards, abT, bbs = _prep(
        x, w_qkv, np.asarray(lora_a), np.asarray(lora_b_q),
        np.asarray(lora_b_k), np.asarray(lora_b_v), perm, tiles, corr,
    )
    in_maps = [
        {"xT": xT, "wT": w_shards[c], "abT": abT, "bbT": bbs[c]}
        for c in range(NCORES)
    ]
    res = bass_utils.run_bass_kernel_spmd(
        nc, in_maps, core_ids=list(range(NCORES))
    )
    LAST_RESULT = res
    out_perm = np.concatenate([res.results[c]["out"] for c in range(NCORES)],
                              axis=1)
    out = np.empty((T, O), np.float32)
    out[perm] = out_perm
    return out


# revision 34
# speedup vs baseline: 1.0032x; 1.0032x over previous
"""Trainium2 Bass kernel: MergedQKVParallelLinearWithLoRA.

out = x @ w_qkv.T + concat_s( lora_expand_s( lora_shrink_s(x)[token's lora] ) )

Strategy (8 NeuronCores, tensor-parallel on the merged QKV output dim):
  - Each core owns 768 of the 6144 output columns.
  - The LoRA is FOLDED into the base weights on the host:
    W_l = w_qkv + B_l @ A_l for each of the 8 loras (bf16).  Tokens are
    sorted by lora id host-side, so the device kernel is a pure grouped
    GEMM: each 128-token tile multiplies with the folded W of its lora.
    Per-group folded weights stream HBM->SBUF (6.3MB bf16 each) with a
    double-buffered pool, prefetched a full group ahead.
  - A 128-token tile that straddles a group boundary is computed with its
    majority segment's W, then corrected with a rank-96 low-rank update:
    ps = [A_wrong; A_main] @ x_tile^T (PE), zero the columns of tokens
    outside the wrong segment, expand with [B_wrong; -B_main].
  - All matmuls in bf16 (1 row/cycle, no fp32r >=256 free-dim constraint),
    fp32 PSUM accumulation; ~2.6e-3 max rel err vs the 2e-2 gate.
  - Cold start is DMA-bound (~150-250GB/s while queues ramp): the first
    two tiles are fused into one shared k-loop (halving W-arrival demand)
    and their x/W chunks are emitted on the sync queue in exact
    consumption order, which also FIFO-gates later-group W prefetches out
    of the starved window.

The kernel is specialized at build time to the token->lora grouping;
`kernel()` re-derives it from token_lora_idx on every call, so it is
correct for arbitrary inputs of the fixed shapes below.
"""

import numpy as np
import ml_dtypes

import concourse.mybir as mybir
import concourse.tile as tile
from concourse import bacc, bass_utils

T, D = 8192, 4096
L, R = 8, 16
OUT_SLICES = (4096, 1024, 1024)
O = sum(OUT_SLICES)          # 6144
NCORES = 8
OS = O // NCORES             # 768 output cols per core
P = 128
KT = D // P                  # 32 k-tiles
NT = T // P                  # 64 token tiles
RC = 3 * R                   # 48 stacked lora-rank rows (q,k,v)
RC2 = 2 * RC                 # 96: [wrong; main] stacked correction rank
N0 = 512                     # matmul free-dim split at the PSUM bank edge

F32 = mybir.dt.float32
BF16 = mybir.dt.bfloat16
NPBF16 = ml_dtypes.bfloat16

LAST_RESULT = None           # BassKernelResults of the most recent run


def _schedule(sorted_idx: np.ndarray):
    """Per-128-token-tile list of (lora, a, b) sub-ranges (a/b rel. to tile)."""
    tiles = []
    for t in range(NT):
        win = sorted_idx[t * P : (t + 1) * P]
        segs = []
        a = 0
        for i in range(1, P + 1):
            if i == P or win[i] != win[a]:
                segs.append((int(win[a]), a, i))
                a = i
        tiles.append(segs)
    return tiles


def _build(tiles, corr):
    mains = [max(segs, key=lambda s: s[2] - s[1])[0] for segs in tiles]
    groups = []                      # (lora, tile_start, tile_end)
    t0 = 0
    for t in range(1, NT + 1):
        if t == NT or mains[t] != mains[t0]:
            groups.append((mains[t0], t0, t))
            t0 = t

    corr_by_tile = {}                # tile -> [(bi, a, b)]
    for bi, (t, a, b, _lw, _lm) in enumerate(corr):
        corr_by_tile.setdefault(t, []).append((bi, a, b))
    nb = max(len(corr), 1)
    maxc = max([len(v) for v in corr_by_tile.values()] + [1])
    cbufs = maxc + 2

    nc = bacc.Bacc("TRN2", target_bir_lowering=False, debug=False,
                   num_devices=NCORES)
    d_x = nc.dram_tensor("xT", [NT, P, KT, P], BF16, kind="ExternalInput")
    d_w = nc.dram_tensor("wT", [L, P, KT, OS], BF16, kind="ExternalInput")
    d_ab = nc.dram_tensor("abT", [nb, P, KT, RC2], BF16, kind="ExternalInput")
    d_bb = nc.dram_tensor("bbT", [nb, RC2, OS], BF16, kind="ExternalInput")
    d_o = nc.dram_tensor("out", [T, OS], BF16, kind="ExternalOutput")

    with tile.TileContext(nc) as tc:
        with (
            tc.tile_pool(name="wpool", bufs=2) as wpool,
            tc.tile_pool(name="xpool", bufs=4) as xpool,
            tc.tile_pool(name="abpool", bufs=cbufs) as abpool,
            tc.tile_pool(name="bbpool", bufs=cbufs) as bbpool,
            tc.tile_pool(name="sbpool", bufs=cbufs) as sbpool,
            tc.tile_pool(name="opool", bufs=3) as opool,
            tc.tile_pool(name="bpsum", bufs=3, space="PSUM") as bpsum,
            tc.tile_pool(name="spsum", bufs=2, space="PSUM") as spsum,
        ):
            corr_tiles = {}          # bi -> (abt, bbt) live SBUF tiles
            xcache = {}              # tile -> prefetched x SBUF tile

            fetched_corr = set()

            def fetch_corrections(t):
                # scalar-engine triggers: never queue behind x/out DMAs
                if t in fetched_corr:
                    return
                fetched_corr.add(t)
                for (bi, _a, _b) in corr_by_tile.get(t, ()):
                    abt = abpool.tile([P, KT, RC2], BF16, tag="ab")
                    nc.scalar.dma_start(abt[:], d_ab[bi])
                    bbt = bbpool.tile([RC2, OS], BF16, tag="bb")
                    nc.scalar.dma_start(bbt[:], d_bb[bi])
                    corr_tiles[bi] = (abt, bbt)

            def ensure_x(t):
                if t < NT and t not in xcache:
                    xtr = xpool.tile([P, KT, P], BF16, tag="xt")
                    nc.sync.dma_start(xtr[:], d_x[t])
                    xcache[t] = xtr

            def base_mm(pb, xtr, wt, k, st, sp):
                nc.tensor.matmul(pb[:, 0:N0], xtr[:, k, :],
                                 wt[:, k, 0:N0], start=st, stop=sp)
                nc.tensor.matmul(pb[:, N0:OS], xtr[:, k, :],
                                 wt[:, k, N0:OS], start=st, stop=sp)

            def drain(pb, t):
                ob = opool.tile([P, OS], BF16, tag="ob")
                nc.vector.tensor_copy(ob[:], pb[:])
                nc.sync.dma_start(d_o[t * P : (t + 1) * P, :], ob[:])

            fetch_corrections(0)
            # Fuse the first n_cold tiles of group 0 into one shared k-loop:
            # each W k-chunk then feeds n_cold x the PE work, matching
            # W-arrival demand to the ~250GB/s cold-start DMA rate while the
            # 6.3MB W0 streams in.
            g0len = groups[0][2] - groups[0][1]
            n_cold = 0
            for n in (2,):
                if g0len >= n and not any(
                    corr_by_tile.get(groups[0][1] + j) for j in range(n)
                ):
                    n_cold = n
                    break
            for gi, (gl, gt0, gt1) in enumerate(groups):
                wt = wpool.tile([P, KT, OS], BF16, tag="wt")
                t = gt0
                if gi == 0 and n_cold:
                    # Cold start: everything on the sync queue in exact
                    # need-order — x chunks for the fused tiles first, W0
                    # k-progressively interleaved. Later-group W DMAs queue
                    # behind this stream, so nothing steals cold bandwidth.
                    xs = [xpool.tile([P, KT, P], BF16, tag="xt",
                                     name=f"xcold_{j}")
                          for j in range(n_cold)]
                    # uniform consumption-order interleave: per round,
                    # [xa, xb, W] k-chunks — arrival tracks the fused
                    # k-loop's demand with no bursts. A small first round
                    # gets the first matmul issued sooner.
                    bounds = (0, 2, 6, 10, 14, 18, 22, 26, 30, KT)
                    for k0, k1 in zip(bounds[:-1], bounds[1:]):
                        for j, xj in enumerate(xs):
                            nc.sync.dma_start(xj[:, k0:k1, :],
                                              d_x[gt0 + j, :, k0:k1, :])
                        nc.sync.dma_start(wt[:, k0:k1, :],
                                          d_w[gl, :, k0:k1, :])
                    for j in range(1, n_cold + 1):
                        fetch_corrections(gt0 + j)
                    pbs = [bpsum.tile([P, OS], F32, tag="pb",
                                      name=f"pbcold_{j}")
                           for j in range(n_cold)]
                    for k in range(KT):
                        st, sp = k == 0, k == KT - 1
                        for xj, pj in zip(xs, pbs):
                            base_mm(pj, xj, wt, k, st, sp)
                    for j, pj in enumerate(pbs):
                        drain(pj, gt0 + j)
                    t = gt0 + n_cold
                elif gi == 0:
                    for k0 in range(0, KT, 4):
                        nc.sync.dma_start(wt[:, k0 : k0 + 4, :],
                                          d_w[gl, :, k0 : k0 + 4, :])
                else:
                    for k0 in range(0, KT, 8):
                        nc.sync.dma_start(wt[:, k0 : k0 + 8, :],
                                          d_w[gl, :, k0 : k0 + 8, :])

                while t < gt1:
                    ensure_x(t)
                    xtr = xcache.pop(t)
                    if t + 1 < NT:
                        fetch_corrections(t + 1)

                    cs = corr_by_tile.get(t, ())
                    # Boundary shrinks first: their DVE zero-pad+copy then
                    # overlaps with the base k-loop below.
                    sbs = []
                    for (bi, a, b) in cs:
                        abt, bbt = corr_tiles.pop(bi)
                        a2, b2 = a & ~1, min(P, (b + 1) & ~1)
                        ps = spsum.tile([RC2, P], F32, tag="ps")
                        for k in range(KT):
                            nc.tensor.matmul(
                                ps[:, a2:b2], abt[:, k, :], xtr[:, k, a2:b2],
                                start=(k == 0), stop=(k == KT - 1),
                            )
                        sb = sbpool.tile([RC2, P], BF16, tag="sb")
                        nc.vector.memset(sb[:], 0.0)
                        nc.vector.tensor_copy(sb[:, a:b], ps[:, a:b])
                        sbs.append((sb, bbt))

                    pb = bpsum.tile([P, OS], F32, tag="pb")
                    last_base = len(sbs) == 0
                    for k in range(KT):
                        base_mm(pb, xtr, wt, k, k == 0,
                                last_base and k == KT - 1)
                    for ci, (sb, bbt) in enumerate(sbs):
                        sp = ci == len(sbs) - 1
                        nc.tensor.matmul(pb[:, 0:N0], sb[:], bbt[:, 0:N0],
                                         start=False, stop=sp)
                        nc.tensor.matmul(pb[:, N0:OS], sb[:], bbt[:, N0:OS],
                                         start=False, stop=sp)

                    if t == NT - 1:
                        # tail trim: half-drains on two DMA queues so the
                        # final HBM write starts while the second PSUM half
                        # is still being copied out.
                        ob = opool.tile([P, OS], BF16, tag="ob")
                        nc.vector.tensor_copy(ob[:, 0:N0], pb[:, 0:N0])
                        nc.sync.dma_start(d_o[t * P : (t + 1) * P, 0:N0],
                                          ob[:, 0:N0])
                        nc.vector.tensor_copy(ob[:, N0:OS], pb[:, N0:OS])
                        nc.gpsimd.dma_start(d_o[t * P : (t + 1) * P, N0:OS],
                                            ob[:, N0:OS])
                    else:
                        drain(pb, t)
                    t += 1

    nc.compile()
    return nc


def _prep(x, w_qkv, lora_a, lora_b_q, lora_b_k, lora_b_v, perm, tiles, corr):
    x = np.asarray(x, dtype=np.float32)
    xs = x[perm]
    # xT[t, p, kt, i] = xs[t*128+i, kt*128+p]
    xT = np.ascontiguousarray(
        xs.T.reshape(KT, P, NT, P).transpose(2, 1, 0, 3)
    ).astype(NPBF16)

    # a_cat[l] = [48, D] (q,k,v stacked); b_cat[l] = [48, O] zero-padded
    a_cat = np.ascontiguousarray(
        np.asarray(lora_a, np.float32).transpose(1, 0, 2, 3)
    ).reshape(L, RC, D)
    b_cat = np.zeros((L, RC, O), np.float32)
    off = 0
    for s, (bs, osz) in enumerate(
        zip((lora_b_q, lora_b_k, lora_b_v), OUT_SLICES)
    ):
        b_cat[:, R * s : R * (s + 1), off : off + osz] = np.asarray(
            bs, np.float32
        ).transpose(0, 2, 1)
        off += osz

    w_qkv = np.asarray(w_qkv, np.float32)
    w_shards = []                    # per core: [L, P, KT, OS] bf16 folded
    for c in range(NCORES):
        wc = w_qkv[c * OS : (c + 1) * OS]            # [OS, D]
        bc = b_cat[:, :, c * OS : (c + 1) * OS]      # [L, 48, OS]
        sh = np.empty((L, P, KT, OS), NPBF16)
        for l in range(L):
            wf = wc + bc[l].T @ a_cat[l]             # [OS, D]
            sh[l] = wf.T.reshape(KT, P, OS).transpose(1, 0, 2).astype(NPBF16)
        w_shards.append(sh)

    nb = max(len(corr), 1)
    abT = np.zeros((nb, P, KT, RC2), NPBF16)
    bbs = [np.zeros((nb, RC2, OS), NPBF16) for _ in range(NCORES)]
    for bi, (t, a, b, lw, lm) in enumerate(corr):
        A2 = np.concatenate([a_cat[lw], a_cat[lm]], axis=0)   # [96, D]
        abT[bi] = A2.T.reshape(KT, P, RC2).transpose(1, 0, 2).astype(NPBF16)
        for c in range(NCORES):
            bbs[c][bi, 0:RC] = b_cat[lw, :, c * OS : (c + 1) * OS].astype(NPBF16)
            bbs[c][bi, RC:RC2] = (-b_cat[lm, :, c * OS : (c + 1) * OS]).astype(NPBF16)
    return xT, w_shards, abT, bbs


def kernel(x, w_qkv, lora_a, lora_b_q, lora_b_k, lora_b_v, token_lora_idx):
    global LAST_RESULT
    idx = np.asarray(token_lora_idx)
    perm = np.argsort(idx, kind="stable")
    tiles = _schedule(idx[perm])
    corr = []                        # (tile, a, b, lora_wrong, lora_main)
    for t, segs in enumerate(tiles):
        lm = max(segs, key=lambda s: s[2] - s[1])[0]
        for (l, a, b) in segs:
            if l != lm:
                corr.append((t, a, b, l, lm))

    nc = _build(tiles, corr)
    xT, w_shards, abT, bbs = _prep(
        x, w_qkv, np.asarray(lora_a), np.asarray(lora_b_q),
        np.asarray(lora_b_k), np.asarray(lora_b_v), perm, tiles, corr,
    )
    in_maps = [
        {"xT": xT, "wT": w_shards[c], "abT": abT, "bbT": bbs[c]}
        for c in range(NCORES)
    ]
    res = bass_utils.run_bass_kernel_spmd(
        nc, in_maps, core_ids=list(range(NCORES))
    )
    LAST_RESULT = res
    out_perm = np.concatenate(
        [np.asarray(res.results[c]["out"]).astype(np.float32)
         for c in range(NCORES)], axis=1)
    out = np.empty((T, O), np.float32)
    out[perm] = out_perm
    return out
